# revision 2
# baseline (speedup 1.0000x reference)
"""Trainium2 Bass kernel for ApplyDF (deep-filtering, order-5 complex FIR over time).

Reference semantics (per example b, time t, band freq f < NB):
    out[b,0,t,f] = sum_{n=0}^{4} coefs[b,n,t,f] * spec[b,0,t+n-4,f]   (complex)
    out[b,0,t,f>=NB] = spec[b,0,t,f]                                  (passthrough)

Sharding: pure data-parallel over batch B=32 across 8 NeuronCores (4 examples
per core). No cross-core communication.

Strategy (v2):
  * The device computes ONLY the filtered 96-bin band. The passthrough bins
    (96..480) never touch the device: the host pastes the filtered band into a
    copy of the input spectrogram. This removes ~2/3 of the HBM traffic.
  * All device tensors are bfloat16 with the re/im planes SPLIT (de-interleaved
    on the host). Unit-stride bf16 tensor_tensor runs in the DVE 2x perf mode
    (2 elem/lane/cycle) vs 1x for fp32 or strided bf16, and halves DMA bytes.
    bf16 rounding contributes ~0.3% relative error (gate is 2e-2).
  * Per-core layout: one frame per example; the 2000 time steps are chunked
    onto 125 SBUF partitions x 16 steps. Within a partition row the (time,
    freq) layout is kept, so the 4 history steps prepend the row and FIR lag
    shifts are contiguous free-dim offsets.
  * SBUF-side loads go through SWDGE (nc.gpsimd): its descriptor swizzle
    spreads transfers across all 16 SDMA engines. Coefficient loads are split
    per lag, issued in compute order. Band stores alternate the two HWDGE
    rings (ACT/SP). Tiny per-DMA "probe" copies on the consuming engine absorb
    completion waits (walrus caps compute instructions at ONE sync wait).
  * Optionally a column slice of the FIR runs on GpSimd (gp_cols) to offload
    the VectorE, which is the critical path once DMA is halved.
"""

import numpy as np
import ml_dtypes

import concourse.bass as bass
import concourse.bacc as bacc
import concourse.mybir as mybir
from concourse import tile
from concourse.bass_utils import run_bass_kernel_spmd

# Problem shapes (hardcoded per spec).
B, T, F, NB, ORDER = 32, 2000, 481, 96, 5
NCORES = 8
BLOC = B // NCORES  # 4 examples per core
HIST = ORDER - 1    # 4 history steps (causal window, LOOKAHEAD=0)

F32 = mybir.dt.float32
BF16 = mybir.dt.bfloat16
NPBF16 = np.dtype(ml_dtypes.bfloat16)


def build_nc(bloc=BLOC, t=T, nb=NB, tc=16, gp_cols=0, bufs=2, tmp_bufs=4):
    """Build the per-core Bass program.

    bloc: examples per core; t: time; nb: filtered band freqs;
    tc: time steps per partition; gp_cols: band columns (of tc*nb per
    partition) computed on GpSimd instead of VectorE.
    """
    assert t % tc == 0
    p = t // tc               # partitions used
    assert p <= 128
    row = nb                  # elems per time step per plane
    srow = (tc + HIST) * row  # S plane elems per partition
    crow = tc * row           # C/O plane elems per partition (one lag)

    nc = bacc.Bacc()
    # Split re/im planes, bf16, band only.
    sb_d = nc.declare_dram_parameter("sb", [bloc, 2, t, nb], BF16, isOutput=False)
    cb_d = nc.declare_dram_parameter(
        "cb", [bloc, ORDER, 2, t, nb], BF16, isOutput=False
    )
    ob_d = nc.declare_dram_parameter("ob", [bloc, 2, t, nb], BF16, isOutput=True)

    ncols = crow              # band output columns per partition per plane
    vcols = ncols - gp_cols   # columns on VectorE
    assert vcols % 2 == 0 and gp_cols % 2 == 0
    with tile.TileContext(nc) as tc_:
        with (
            tc_.tile_pool(name="s", bufs=bufs) as s_pool,
            tc_.tile_pool(name="c", bufs=bufs) as c_pool,
            tc_.tile_pool(name="o", bufs=bufs + 1) as o_pool,
            tc_.tile_pool(name="tmp", bufs=tmp_bufs) as tmp_pool,
        ):
            ld = nc.gpsimd

            for b in range(bloc):
                S = s_pool.tile([p, 2 * srow], BF16, tag="S")
                C = c_pool.tile([p, ORDER * 2 * crow], BF16, tag="C")
                O = o_pool.tile([p, 2 * crow], BF16, tag="O")

                # Main band rows: partition q gets times tc*q .. tc*q+tc-1,
                # both planes, at row offset HIST*row.
                main_src = sb_d[b].rearrange("pl (q j) f -> q pl j f", j=tc)
                main_dst = S[:].rearrange("q (pl x) -> q pl x", pl=2)[
                    :, :, HIST * row :
                ].rearrange("q pl (j f) -> q pl j f", j=tc)
                ld.dma_start(out=main_dst, in_=main_src)

                # History rows: partition q>=1 needs steps q*tc-4 .. q*tc-1.
                # Partition 0's history is zero (causal pad).
                nc.vector.memset(
                    S[0:1].rearrange("q (pl x) -> q pl x", pl=2)[:, :, : HIST * row],
                    0.0,
                )
                hist_src = sb_d[b][:, tc - HIST : tc - HIST + (p - 1) * tc, :].rearrange(
                    "pl (q j) f -> q pl j f", j=tc
                )[:, :, :HIST]
                hist_dst = S[1:p].rearrange("q (pl x) -> q pl x", pl=2)[
                    :, :, : HIST * row
                ].rearrange("q pl (j f) -> q pl j f", j=HIST)
                ld.dma_start(out=hist_dst, in_=hist_src)

                # C loads split per lag, in compute order (n = 4 .. 0):
                # the lag-4 products can start as soon as its chunk lands.
                csrc = cb_d[b].rearrange("n pl (q j) f -> q n pl j f", j=tc)
                cdst = C[:].rearrange(
                    "q (n pl j f) -> q n pl j f", n=ORDER, pl=2, j=tc
                )
                for n in range(ORDER - 1, -1, -1):
                    ld.dma_start(out=cdst[:, n], in_=csrc[:, n])

                # Sync probes: walrus caps sync-waits at ONE per compute
                # instruction, so absorb each DMA-completion (and the O-buffer
                # release) into its own tiny op per consuming engine.
                for ei, (eng, active) in enumerate(
                    ((nc.vector, vcols), (nc.gpsimd, gp_cols))
                ):
                    if active == 0:
                        continue
                    p2 = tmp_pool.tile([1, 2], BF16, tag=f"pr2_{ei}")
                    eng.tensor_copy(p2[:], S[0:1, HIST * row : HIST * row + 2])
                    eng.memset(O[0:1, 2 * ei : 2 * ei + 2], 0.0)

                # Complex FIR over the 5 lags; all ops unit-stride bf16 2x.
                # Lags run n=4 -> 0: lag 4 reads only the main S region
                # (no history rows) and initializes O via direct products.
                Oe, Oi = O[:, 0:crow], O[:, crow : 2 * crow]
                for n in range(ORDER - 1, -1, -1):
                    Se = S[:, n * row : n * row + crow]
                    Si = S[:, srow + n * row : srow + n * row + crow]
                    Ce = C[:, (2 * n) * crow : (2 * n + 1) * crow]
                    Ci = C[:, (2 * n + 1) * crow : (2 * n + 2) * crow]
                    for ei, (eng, c0, cn) in enumerate(
                        ((nc.vector, 0, vcols), (nc.gpsimd, vcols, gp_cols))
                    ):
                        if cn == 0:
                            continue
                        # per-chunk sync probe for this lag's C data
                        p3 = tmp_pool.tile([1, 2], BF16, tag=f"pr3_{ei}")
                        eng.tensor_copy(
                            p3[:], C[0:1, 2 * n * crow : 2 * n * crow + 2]
                        )
                        cs = slice(c0, c0 + cn)
                        oe, oi = Oe[:, cs], Oi[:, cs]
                        se, si = Se[:, cs], Si[:, cs]
                        ce, ci = Ce[:, cs], Ci[:, cs]
                        t1 = tmp_pool.tile([p, cn], BF16, tag=f"t1_{c0}")
                        t2 = tmp_pool.tile([p, cn], BF16, tag=f"t2_{c0}")
                        if n == ORDER - 1:
                            eng.tensor_mul(oe, ce, se)
                            eng.tensor_mul(t1[:], ci, si)
                            eng.tensor_sub(oe, oe, t1[:])
                            eng.tensor_mul(oi, ce, si)
                            eng.tensor_mul(t2[:], ci, se)
                            eng.tensor_add(oi, oi, t2[:])
                        else:
                            eng.tensor_mul(t1[:], ce, se)
                            eng.tensor_add(oe, oe, t1[:])
                            eng.tensor_mul(t1[:], ci, si)
                            eng.tensor_sub(oe, oe, t1[:])
                            eng.tensor_mul(t2[:], ce, si)
                            eng.tensor_add(oi, oi, t2[:])
                            eng.tensor_mul(t2[:], ci, se)
                            eng.tensor_add(oi, oi, t2[:])

                # Band store rides the HWDGE rings, alternating to split the
                # per-ring burden; keeps the SWDGE FIFO pure loads.
                odst = ob_d[b].rearrange("pl (q j) f -> q pl j f", j=tc)
                osrc = O[:].rearrange("q (pl j f) -> q pl j f", pl=2, j=tc)
                (nc.scalar if b % 2 == 0 else nc.sync).dma_start(out=odst, in_=osrc)

    nc.compile()
    return nc


_NC_CACHE = {}


def _get_nc(**kwargs):
    key = tuple(sorted(kwargs.items()))
    if key not in _NC_CACHE:
        _NC_CACHE[key] = build_nc(**kwargs)
    return _NC_CACHE[key]


def _prep_inputs(spec, coefs):
    """Host-side: slice band, de-interleave re/im, cast to bf16."""
    # spec: [B,1,T,F,2] f32 -> sb [B,2,T,NB] bf16
    band = spec[:, 0, :, :NB, :]                      # [B,T,NB,2] view
    sb = np.ascontiguousarray(np.moveaxis(band, 3, 1)).astype(NPBF16)
    # coefs: [B,ORDER,T,NB,2] f32 -> cb [B,ORDER,2,T,NB] bf16
    cb = np.ascontiguousarray(np.moveaxis(coefs, 4, 2)).astype(NPBF16)
    return sb, cb


def run(spec, coefs, trace=False, **build_kwargs):
    """Run the SPMD kernel on 8 cores. Returns (out, BassKernelResults)."""
    spec = np.ascontiguousarray(spec, dtype=np.float32)
    coefs = np.ascontiguousarray(coefs, dtype=np.float32)
    sb, cb = _prep_inputs(spec, coefs)
    nc = _get_nc(**build_kwargs)
    in_maps = []
    for i in range(NCORES):
        sl = slice(i * BLOC, (i + 1) * BLOC)
        in_maps.append({"sb": sb[sl], "cb": cb[sl]})
    r = run_bass_kernel_spmd(nc, in_maps, list(range(NCORES)), trace=trace)
    ob = np.concatenate([r.results[i]["ob"] for i in range(NCORES)], axis=0)
    # Paste the filtered band into a copy of the full input spectrogram.
    out = spec.copy()
    band = np.asarray(ob).astype(np.float32)          # [B,2,T,NB]
    out[:, 0, :, :NB, 0] = band[:, 0]
    out[:, 0, :, :NB, 1] = band[:, 1]
    return out, r


def kernel(spec, coefs):
    out, _ = run(spec, coefs)
    return out


# revision 3
# speedup vs baseline: 1.0215x; 1.0215x over previous
"""Trainium2 Bass kernel for ApplyDF (deep-filtering, order-5 complex FIR over time).

Reference semantics (per example b, time t, band freq f < NB):
    out[b,0,t,f] = sum_{n=0}^{4} coefs[b,n,t,f] * spec[b,0,t+n-4,f]   (complex)
    out[b,0,t,f>=NB] = spec[b,0,t,f]                                  (passthrough)

Sharding: pure data-parallel over batch B=32 across 8 NeuronCores (4 examples
per core). No cross-core communication.

Strategy (v3):
  * The device computes ONLY the filtered 96-bin band. The passthrough bins
    (96..480) never touch the device: the host pastes the filtered band into a
    copy of the input spectrogram. This removes ~2/3 of the HBM traffic.
  * All device tensors are bfloat16 with the re/im planes SPLIT (de-interleaved
    on the host). Unit-stride bf16 tensor_tensor runs in the DVE 2x perf mode
    (2 elem/lane/cycle) vs 1x for fp32 or strided bf16, and halves DMA bytes.
    bf16 rounding contributes ~0.4% relative error (gate is 2e-2).
  * PARTITION-MAJOR DRAM layout, packed on the host: the upload buffers are
    ordered [example, partition, ...] so each partition's whole frame payload
    is one contiguous DRAM run. v2's time-major layout produced 3 KB DMA
    descriptors that ran the SDMA engines at ~13 GB/s; partition-major gives
    6-30 KB descriptors near line rate. The 4 FIR history steps are
    replicated into each partition's row on the host (+25% band bytes, but
    no separate history DMA, no memset, no partition-0 edge case).
  * Per-core layout: one frame per example; 2000 time steps chunked onto 125
    SBUF partitions x 16 steps (+4 history). FIR lag shifts are contiguous
    free-dim offsets within each partition row.
  * SBUF-side loads ride SWDGE (nc.gpsimd), which spreads descriptors across
    all 16 SDMA engines; coefficient loads are split per lag in compute
    order so lag-4 products start as soon as that chunk lands. Band stores
    alternate the two HWDGE rings. Tiny per-DMA "probe" copies on the
    consuming engine absorb completion waits (walrus caps compute
    instructions at ONE sync wait).
  * Optionally a column slice of the FIR runs on GpSimd (gp_cols) to offload
    the VectorE, which is the critical path once DMA is fixed.
"""

import numpy as np
import ml_dtypes

import concourse.bass as bass
import concourse.bacc as bacc
import concourse.mybir as mybir
from concourse import tile
from concourse.bass_utils import run_bass_kernel_spmd

# Problem shapes (hardcoded per spec).
B, T, F, NB, ORDER = 32, 2000, 481, 96, 5
NCORES = 8
BLOC = B // NCORES  # 4 examples per core
HIST = ORDER - 1    # 4 history steps (causal window, LOOKAHEAD=0)
TC = 16             # time steps per partition
P = T // TC         # 125 partitions

F32 = mybir.dt.float32
BF16 = mybir.dt.bfloat16
NPBF16 = np.dtype(ml_dtypes.bfloat16)


def build_nc(bloc=BLOC, t=T, nb=NB, tc=TC, gp_cols=0, bufs=2, tmp_bufs=4):
    """Build the per-core Bass program."""
    assert t % tc == 0
    p = t // tc               # partitions used
    assert p <= 128
    row = nb                  # elems per time step per plane
    srow = (tc + HIST) * row  # S plane elems per partition
    crow = tc * row           # C/O plane elems per partition per lag

    nc = bacc.Bacc()
    # Partition-major, split re/im planes, bf16, band only.
    sb_d = nc.declare_dram_parameter(
        "sb", [bloc, p, 2, tc + HIST, nb], BF16, isOutput=False
    )
    cb_d = nc.declare_dram_parameter(
        "cb", [bloc, ORDER, p, 2, tc, nb], BF16, isOutput=False
    )
    ob_d = nc.declare_dram_parameter("ob", [bloc, p, 2, tc, nb], BF16, isOutput=True)

    ncols = crow              # band output columns per partition per plane
    vcols = ncols - gp_cols   # columns on VectorE
    assert vcols % 2 == 0 and gp_cols % 2 == 0
    with tile.TileContext(nc) as tc_:
        with (
            tc_.tile_pool(name="s", bufs=bufs) as s_pool,
            tc_.tile_pool(name="c", bufs=bufs) as c_pool,
            tc_.tile_pool(name="o", bufs=bufs + 1) as o_pool,
            tc_.tile_pool(name="tmp", bufs=tmp_bufs) as tmp_pool,
        ):
            ld = nc.gpsimd

            for b in range(bloc):
                S = s_pool.tile([p, 2 * srow], BF16, tag="S")
                C = c_pool.tile([p, ORDER * 2 * crow], BF16, tag="C")
                O = o_pool.tile([p, 2 * crow], BF16, tag="O")

                # One contiguous run per partition (history pre-replicated).
                ld.dma_start(
                    out=S[:], in_=sb_d[b].rearrange("q pl j f -> q (pl j f)")
                )
                # C loads split per lag, in compute order (n = 4 .. 0).
                cdst = C[:].rearrange("q (n x) -> q n x", n=ORDER)
                for n in range(ORDER - 1, -1, -1):
                    ld.dma_start(
                        out=cdst[:, n],
                        in_=cb_d[b, n].rearrange("q pl j f -> q (pl j f)"),
                    )

                # Sync probes: walrus caps sync-waits at ONE per compute
                # instruction, so absorb the S-DMA completion (and the O-buffer
                # release) into a tiny op per consuming engine.
                for ei, (eng, active) in enumerate(
                    ((nc.vector, vcols), (nc.gpsimd, gp_cols))
                ):
                    if active == 0:
                        continue
                    p2 = tmp_pool.tile([1, 2], BF16, tag=f"pr2_{ei}")
                    eng.tensor_copy(p2[:], S[0:1, 0:2])
                    eng.memset(O[0:1, 2 * ei : 2 * ei + 2], 0.0)

                # Complex FIR over the 5 lags; all ops unit-stride bf16 2x.
                # Lags run n=4 -> 0; lag 4 initializes O via direct products.
                Oe, Oi = O[:, 0:crow], O[:, crow : 2 * crow]
                for n in range(ORDER - 1, -1, -1):
                    Se = S[:, n * row : n * row + crow]
                    Si = S[:, srow + n * row : srow + n * row + crow]
                    Ce = C[:, (2 * n) * crow : (2 * n + 1) * crow]
                    Ci = C[:, (2 * n + 1) * crow : (2 * n + 2) * crow]
                    for ei, (eng, c0, cn) in enumerate(
                        ((nc.vector, 0, vcols), (nc.gpsimd, vcols, gp_cols))
                    ):
                        if cn == 0:
                            continue
                        # per-chunk sync probe for this lag's C data
                        p3 = tmp_pool.tile([1, 2], BF16, tag=f"pr3_{ei}")
                        eng.tensor_copy(
                            p3[:], C[0:1, 2 * n * crow : 2 * n * crow + 2]
                        )
                        cs = slice(c0, c0 + cn)
                        oe, oi = Oe[:, cs], Oi[:, cs]
                        se, si = Se[:, cs], Si[:, cs]
                        ce, ci = Ce[:, cs], Ci[:, cs]
                        t1 = tmp_pool.tile([p, cn], BF16, tag=f"t1_{c0}")
                        t2 = tmp_pool.tile([p, cn], BF16, tag=f"t2_{c0}")
                        if n == ORDER - 1:
                            eng.tensor_mul(oe, ce, se)
                            eng.tensor_mul(t1[:], ci, si)
                            eng.tensor_sub(oe, oe, t1[:])
                            eng.tensor_mul(oi, ce, si)
                            eng.tensor_mul(t2[:], ci, se)
                            eng.tensor_add(oi, oi, t2[:])
                        else:
                            eng.tensor_mul(t1[:], ce, se)
                            eng.tensor_add(oe, oe, t1[:])
                            eng.tensor_mul(t1[:], ci, si)
                            eng.tensor_sub(oe, oe, t1[:])
                            eng.tensor_mul(t2[:], ce, si)
                            eng.tensor_add(oi, oi, t2[:])
                            eng.tensor_mul(t2[:], ci, se)
                            eng.tensor_add(oi, oi, t2[:])

                # Band store rides the HWDGE rings, alternating to split the
                # per-ring burden; keeps the SWDGE FIFO pure loads.
                (nc.scalar if b % 2 == 0 else nc.sync).dma_start(
                    out=ob_d[b].rearrange("q pl j f -> q (pl j f)"), in_=O[:]
                )

    nc.compile()
    return nc


_NC_CACHE = {}


def _get_nc(**kwargs):
    key = tuple(sorted(kwargs.items()))
    if key not in _NC_CACHE:
        _NC_CACHE[key] = build_nc(**kwargs)
    return _NC_CACHE[key]


def _prep_inputs(spec, coefs):
    """Host-side: slice band, de-interleave re/im, partition-major pack,
    replicate FIR history, cast to bf16."""
    # spec band -> sb [B, P, 2, HIST+TC, NB] bf16 (with per-partition history)
    band = spec[:, 0, :, :NB, :]                      # [B,T,NB,2] view
    padded = np.zeros((B, HIST + T, NB, 2), dtype=np.float32)
    padded[:, HIST:] = band
    s0, s1, s2, s3 = padded.strides
    win = np.lib.stride_tricks.as_strided(
        padded, shape=(B, P, HIST + TC, NB, 2), strides=(s0, TC * s1, s1, s2, s3)
    )
    sb = win.transpose(0, 1, 4, 2, 3).astype(NPBF16)  # [B,P,2,HIST+TC,NB]
    # coefs -> cb [B, ORDER, P, 2, TC, NB] bf16
    cw = coefs.reshape(B, ORDER, P, TC, NB, 2)
    cb = cw.transpose(0, 1, 2, 5, 3, 4).astype(NPBF16)
    return sb, cb


def run(spec, coefs, trace=False, **build_kwargs):
    """Run the SPMD kernel on 8 cores. Returns (out, BassKernelResults)."""
    spec = np.ascontiguousarray(spec, dtype=np.float32)
    coefs = np.ascontiguousarray(coefs, dtype=np.float32)
    sb, cb = _prep_inputs(spec, coefs)
    nc = _get_nc(**build_kwargs)
    in_maps = []
    for i in range(NCORES):
        sl = slice(i * BLOC, (i + 1) * BLOC)
        in_maps.append({"sb": sb[sl], "cb": cb[sl]})
    r = run_bass_kernel_spmd(nc, in_maps, list(range(NCORES)), trace=trace)
    ob = np.concatenate([r.results[i]["ob"] for i in range(NCORES)], axis=0)
    # Paste the filtered band into a copy of the full input spectrogram.
    out = spec.copy()
    band = np.asarray(ob)                             # [B,P,2,TC,NB] bf16
    out[:, 0, :, :NB, :] = (
        band.transpose(0, 1, 3, 4, 2).reshape(B, T, NB, 2).astype(np.float32)
    )
    return out, r


def kernel(spec, coefs):
    out, _ = run(spec, coefs)
    return out
